# revision 3
# baseline (speedup 1.0000x reference)
"""Local causal (sliding-window) attention kernel for Trainium2, SPMD over 8 cores.

Problem: states [4, 4096, 1024] f32; q/k/v = states @ W*.T + b*; each query t
attends keys t-8..t (window=8), softmax over valid positions, out = attn @ v.

Sharding: data-parallel, 8 shards = 4 batches x 2 sequence halves (2048 queries
each). The host supplies each shard's states pre-transposed to [H, 2056] with an
8-row halo (zero-padded at sequence start; masked out via the additive mask).

Per-core plan (all matmuls bf16 on TensorE, f32 PSUM accumulate):
  - qT[H,2048], kT[H,2056] in transposed layout; v[2056,H] row-major. All three
    projections consume the same X = states_shard.T; bias for q/k is added on
    the PSUM->SBUF copy (per-partition bias on ScalarE); bias for v via a DVE
    tensor_add against a host-broadcast [128,H] bias tile. 1/sqrt(H) is folded
    into Wq/bq on the host.
  - Attention per 128-query tile j: S[128,136] = Q_j^T.T @ K_span (8 accum
    matmuls over H chunks); additive band mask; softmax along free dim with
    fused exp+rowsum; P transposed via PE identity matmul; out = P @ V_span
    (136-contraction split 128+8); 1/rowsum applied on the PSUM->SBUF copy.
"""

import numpy as np
import ml_dtypes

import concourse.bacc as bacc
import concourse.mybir as mybir
import concourse.tile as tile
from concourse.bass_utils import run_bass_kernel_spmd

B, T, H = 4, 4096, 1024
NCORES = 8
TC = T // 2            # queries per core
HALO = 8               # window size
TH = TC + HALO         # shard rows incl. halo
SPAN = 128 + HALO      # key span per 128-query tile
NT = TC // 128         # query tiles per core
HC = H // 128          # 128-row chunks of H
F32 = mybir.dt.float32
BF16 = mybir.dt.bfloat16
BF = ml_dtypes.bfloat16
AF = mybir.ActivationFunctionType

_cache = {}


def _build():
    if "nc" in _cache:
        return _cache["nc"]
    nc = bacc.Bacc("TRN2", target_bir_lowering=False, debug=False,
                   num_devices=NCORES)

    x_d = nc.dram_tensor("x", [H, TH], BF16, kind="ExternalInput").ap()
    wq_d = nc.dram_tensor("wq", [H, H], BF16, kind="ExternalInput").ap()
    wk_d = nc.dram_tensor("wk", [H, H], BF16, kind="ExternalInput").ap()
    wv_d = nc.dram_tensor("wv", [H, H], BF16, kind="ExternalInput").ap()
    bq_d = nc.dram_tensor("bq", [128, HC], F32, kind="ExternalInput").ap()
    bk_d = nc.dram_tensor("bk", [128, HC], F32, kind="ExternalInput").ap()
    bv_d = nc.dram_tensor("bv", [128, H], F32, kind="ExternalInput").ap()
    m0_d = nc.dram_tensor("m0", [128, SPAN], F32, kind="ExternalInput").ap()
    mr_d = nc.dram_tensor("mr", [128, SPAN], F32, kind="ExternalInput").ap()
    id_d = nc.dram_tensor("ident", [128, 128], BF16, kind="ExternalInput").ap()
    out_d = nc.dram_tensor("out", [TC, H], F32, kind="ExternalOutput").ap()

    with tile.TileContext(nc) as tc:
        with (
            tc.tile_pool(name="consts", bufs=1) as consts,
            tc.tile_pool(name="xw", bufs=1) as xw,
            tc.tile_pool(name="acts", bufs=1) as acts,
            tc.tile_pool(name="psP", bufs=2, space="PSUM") as psP,
            tc.tile_pool(name="psS", bufs=2, space="PSUM") as psS,
            tc.tile_pool(name="psT", bufs=1, space="PSUM") as psT,
            tc.tile_pool(name="psO", bufs=2, space="PSUM") as psO,
            tc.tile_pool(name="attn", bufs=3) as attn,
        ):
            bq_t = consts.tile([128, HC], F32, tag="bq")
            bk_t = consts.tile([128, HC], F32, tag="bk")
            bv_t = consts.tile([128, H], F32, tag="bv")
            m0_t = consts.tile([128, SPAN], F32, tag="m0")
            mr_t = consts.tile([128, SPAN], F32, tag="mr")
            id_t = consts.tile([128, 128], BF16, tag="id")
            nc.sync.dma_start(bq_t[:], bq_d[:])
            nc.sync.dma_start(bk_t[:], bk_d[:])
            nc.sync.dma_start(bv_t[:], bv_d[:])
            nc.sync.dma_start(m0_t[:], m0_d[:])
            nc.sync.dma_start(mr_t[:], mr_d[:])
            nc.sync.dma_start(id_t[:], id_d[:])

            xt = [xw.tile([128, TH], BF16, tag=f"x{c}", name=f"x{c}") for c in range(HC)]
            wqt = [xw.tile([128, H], BF16, tag=f"wq{c}", name=f"wq{c}") for c in range(HC)]
            wkt = [xw.tile([128, H], BF16, tag=f"wk{c}", name=f"wk{c}") for c in range(HC)]
            wvt = [xw.tile([128, H], BF16, tag=f"wv{c}", name=f"wv{c}") for c in range(HC)]
            for c in range(HC):
                nc.sync.dma_start(xt[c][:], x_d[c * 128:(c + 1) * 128, :])
                nc.sync.dma_start(wqt[c][:], wq_d[c * 128:(c + 1) * 128, :])
                nc.sync.dma_start(wkt[c][:], wk_d[c * 128:(c + 1) * 128, :])
                nc.sync.dma_start(wvt[c][:], wv_d[c * 128:(c + 1) * 128, :])

            qt = [acts.tile([128, TC], BF16, tag=f"q{c}", name=f"q{c}") for c in range(HC)]
            kt = [acts.tile([128, TH], BF16, tag=f"k{c}", name=f"k{c}") for c in range(HC)]
            vt = [acts.tile([128, H], BF16, tag=f"v{j}", name=f"v{j}") for j in range(NT + 1)]

            # ---- projections ----
            for hc in range(HC):                      # qT [H, 2048]
                for t4 in range(TC // 512):
                    ps = psP.tile([128, 512], F32, tag="ps")
                    for c in range(HC):
                        nc.tensor.matmul(
                            ps[:], wqt[c][:, hc * 128:(hc + 1) * 128],
                            xt[c][:, HALO + t4 * 512: HALO + (t4 + 1) * 512],
                            start=(c == 0), stop=(c == HC - 1))
                    nc.scalar.activation(
                        qt[hc][:, t4 * 512:(t4 + 1) * 512], ps[:],
                        AF.Identity, bias=bq_t[:, hc:hc + 1], scale=1.0)

            kchunks = [(i * 512, 512) for i in range(TC // 512)] + [(TC, HALO)]
            for hc in range(HC):                      # kT [H, 2056]
                for (off, sz) in kchunks:
                    ps = psP.tile([128, 512], F32, tag="ps")
                    for c in range(HC):
                        nc.tensor.matmul(
                            ps[:, :sz], wkt[c][:, hc * 128:(hc + 1) * 128],
                            xt[c][:, off:off + sz],
                            start=(c == 0), stop=(c == HC - 1))
                    nc.scalar.activation(
                        kt[hc][:, off:off + sz], ps[:, :sz],
                        AF.Identity, bias=bk_t[:, hc:hc + 1], scale=1.0)

            for j in range(NT + 1):                   # v [2056, H] row-major
                pr = 128 if j < NT else HALO
                for hh in range(2):
                    ps = psP.tile([128, 512], F32, tag="ps")
                    for c in range(HC):
                        nc.tensor.matmul(
                            ps[:pr, :], xt[c][:, j * 128: j * 128 + pr],
                            wvt[c][:, hh * 512:(hh + 1) * 512],
                            start=(c == 0), stop=(c == HC - 1))
                    nc.vector.tensor_add(
                        vt[j][:pr, hh * 512:(hh + 1) * 512], ps[:pr, :],
                        bv_t[:pr, hh * 512:(hh + 1) * 512])

            # ---- windowed attention ----
            for j in range(NT):
                s_ps = psS.tile([128, SPAN], F32, tag="s")
                for c in range(HC):
                    nc.tensor.matmul(
                        s_ps[:], qt[c][:, j * 128:(j + 1) * 128],
                        kt[c][:, j * 128: j * 128 + SPAN],
                        start=(c == 0), stop=(c == HC - 1))
                s_sb = attn.tile([128, SPAN], F32, tag="ssb")
                nc.vector.tensor_add(s_sb[:], s_ps[:],
                                     (m0_t if j == 0 else mr_t)[:])
                negmax = attn.tile([128, 1], F32, tag="nm")
                nc.vector.reduce_max(negmax[:], s_sb[:],
                                     axis=mybir.AxisListType.X, negate=True)
                p_bf = attn.tile([128, SPAN], BF16, tag="p")
                rowsum = attn.tile([128, 1], F32, tag="rs")
                nc.scalar.activation(p_bf[:], s_sb[:], AF.Exp,
                                     bias=negmax[:], scale=1.0,
                                     accum_out=rowsum[:])
                rinv = attn.tile([128, 1], F32, tag="ri")
                nc.vector.reciprocal(rinv[:], rowsum[:])

                pta_ps = psT.tile([128, 128], BF16, tag="pta")
                ptb_ps = psT.tile([HALO, 128], BF16, tag="ptb")
                nc.tensor.transpose(pta_ps[:], p_bf[:, 0:128], id_t[:])
                nc.tensor.transpose(ptb_ps[:], p_bf[:, 128:SPAN], id_t[:])
                pta_sb = attn.tile([128, 128], BF16, tag="ptas")
                ptb_sb = attn.tile([HALO, 128], BF16, tag="ptbs")
                nc.scalar.copy(pta_sb[:], pta_ps[:])
                nc.vector.tensor_copy(ptb_sb[:], ptb_ps[:])

                out_sb = attn.tile([128, H], F32, tag="osb")
                for hh in range(2):
                    o_ps = psO.tile([128, 512], F32, tag="o")
                    nc.tensor.matmul(o_ps[:], pta_sb[:],
                                     vt[j][:, hh * 512:(hh + 1) * 512],
                                     start=True, stop=False)
                    nc.tensor.matmul(o_ps[:], ptb_sb[:],
                                     vt[j + 1][:HALO, hh * 512:(hh + 1) * 512],
                                     start=False, stop=True)
                    nc.scalar.activation(
                        out_sb[:, hh * 512:(hh + 1) * 512], o_ps[:],
                        AF.Copy, bias=0.0, scale=rinv[:])
                nc.sync.dma_start(out_d[j * 128:(j + 1) * 128, :], out_sb[:])

    nc.compile()
    _cache["nc"] = nc
    return nc


def _host_inputs(states, Wq, bq, Wk, bk, Wv, bv):
    """Shared (per-run) host-side tensor prep."""
    scale = 1.0 / np.sqrt(H)
    wq_h = np.ascontiguousarray(np.asarray(Wq, np.float32).T * scale).astype(BF)
    wk_h = np.ascontiguousarray(np.asarray(Wk, np.float32).T).astype(BF)
    wv_h = np.ascontiguousarray(np.asarray(Wv, np.float32).T).astype(BF)
    bq_h = np.ascontiguousarray(
        (np.asarray(bq, np.float32) * scale).reshape(HC, 128).T)
    bk_h = np.ascontiguousarray(np.asarray(bk, np.float32).reshape(HC, 128).T)
    bv_h = np.ascontiguousarray(
        np.broadcast_to(np.asarray(bv, np.float32), (128, H)))
    m = np.arange(128)[:, None]
    n = np.arange(SPAN)[None, :]
    band = (n >= m) & (n <= m + HALO)
    mr_h = np.where(band, 0.0, -30000.0).astype(np.float32)
    m0_h = np.where(band & (n >= HALO), 0.0, -30000.0).astype(np.float32)
    id_h = np.eye(128).astype(BF)
    return wq_h, wk_h, wv_h, bq_h, bk_h, bv_h, m0_h, mr_h, id_h


def kernel(states, Wq, bq, Wk, bk, Wv, bv, window):
    assert int(window) == HALO
    states = np.asarray(states, np.float32)
    nc = _build()
    wq_h, wk_h, wv_h, bq_h, bk_h, bv_h, m0_h, mr_h, id_h = _host_inputs(
        states, Wq, bq, Wk, bk, Wv, bv)

    in_maps = []
    for i in range(NCORES):
        b, hf = i // 2, i % 2
        xs = np.zeros((TH, H), np.float32)
        if hf == 0:
            xs[HALO:] = states[b, 0:TC]
        else:
            xs[:] = states[b, TC - HALO: 2 * TC]
        x_h = np.ascontiguousarray(xs.T).astype(BF)
        in_maps.append({
            "x": x_h, "wq": wq_h, "wk": wk_h, "wv": wv_h,
            "bq": bq_h, "bk": bk_h, "bv": bv_h,
            "m0": (m0_h if hf == 0 else mr_h), "mr": mr_h, "ident": id_h,
        })

    res = run_bass_kernel_spmd(nc, in_maps, list(range(NCORES)))
    out = np.empty((B, T, H), np.float32)
    for i in range(NCORES):
        b, hf = i // 2, i % 2
        out[b, hf * TC:(hf + 1) * TC] = res.results[i]["out"]
    return out


# revision 4
# speedup vs baseline: 52.1341x; 52.1341x over previous
"""Local causal (sliding-window) attention kernel for Trainium2, SPMD over 8 cores.

Problem: states [4, 4096, 1024] f32; q/k/v = states @ W*.T + b*; each query t
attends keys t-8..t (window=8), softmax over valid positions, out = attn @ v.

Sharding: data-parallel, 8 shards = 4 batches x 2 sequence halves (2048 queries
each). The host supplies each shard's states pre-transposed to [H, 2056] with an
8-row halo (zero-padded at sequence start; masked out via the additive mask).

Per-core plan (all matmuls bf16 on TensorE, f32 PSUM accumulate):
  - qT[H,2048], kT[H,2056] in transposed layout; v[2056,H] row-major. All three
    projections consume the same X = states_shard.T; bias for q/k is added on
    the PSUM->SBUF copy (per-partition bias on ScalarE); bias for v via a DVE
    tensor_add against a host-broadcast [128,H] bias tile. 1/sqrt(H) is folded
    into Wq/bq on the host.
  - Attention per 128-query tile j: S[128,136] = Q_j^T.T @ K_span (8 accum
    matmuls over H chunks); additive band mask; softmax along free dim with
    fused exp+rowsum; P transposed via PE identity matmul; out = P @ V_span
    (136-contraction split 128+8); 1/rowsum applied on the PSUM->SBUF copy.
"""

import numpy as np
import ml_dtypes

import concourse.bacc as bacc
import concourse.mybir as mybir
import concourse.tile as tile
from concourse.bass_utils import run_bass_kernel_spmd

B, T, H = 4, 4096, 1024
NCORES = 8
TC = T // 2            # queries per core
HALO = 8               # window size
TH = TC + HALO         # shard rows incl. halo
SPAN = 128 + HALO      # key span per 128-query tile
NT = TC // 128         # query tiles per core
HC = H // 128          # 128-row chunks of H
F32 = mybir.dt.float32
BF16 = mybir.dt.bfloat16
BF = ml_dtypes.bfloat16
AF = mybir.ActivationFunctionType

_cache = {}


def _emit(nc, tc, aps, pools):
    (x_d, wq_d, wk_d, wv_d, bq_d, bk_d, bv_d, m0_d, mr_d, id_d, out_d) = aps
    consts, xw, acts, psP, psS, psT, psO, attn = pools

    bq_t = consts.tile([128, HC], F32, tag="bq", name="bq_t")
    bk_t = consts.tile([128, HC], F32, tag="bk", name="bk_t")
    bv_t = consts.tile([128, H], F32, tag="bv", name="bv_t")
    m0_t = consts.tile([128, SPAN], F32, tag="m0", name="m0_t")
    mr_t = consts.tile([128, SPAN], F32, tag="mr", name="mr_t")
    id_t = consts.tile([128, 128], BF16, tag="id", name="id_t")
    nc.sync.dma_start(bq_t[:], bq_d[:])
    nc.sync.dma_start(bk_t[:], bk_d[:])
    nc.sync.dma_start(bv_t[:], bv_d[:])
    nc.sync.dma_start(m0_t[:], m0_d[:])
    nc.sync.dma_start(mr_t[:], mr_d[:])
    nc.sync.dma_start(id_t[:], id_d[:])

    xt = [xw.tile([128, TH], BF16, tag=f"x{c}", name=f"x{c}") for c in range(HC)]
    wqt = [xw.tile([128, H], BF16, tag=f"wq{c}", name=f"wq{c}") for c in range(HC)]
    wkt = [xw.tile([128, H], BF16, tag=f"wk{c}", name=f"wk{c}") for c in range(HC)]
    wvt = [xw.tile([128, H], BF16, tag=f"wv{c}", name=f"wv{c}") for c in range(HC)]
    for c in range(HC):
        nc.sync.dma_start(xt[c][:], x_d[c * 128:(c + 1) * 128, :])
        nc.sync.dma_start(wqt[c][:], wq_d[c * 128:(c + 1) * 128, :])
        nc.sync.dma_start(wkt[c][:], wk_d[c * 128:(c + 1) * 128, :])
        nc.sync.dma_start(wvt[c][:], wv_d[c * 128:(c + 1) * 128, :])

    qt = [acts.tile([128, TC], BF16, tag=f"q{c}", name=f"q{c}") for c in range(HC)]
    kt = [acts.tile([128, TH], BF16, tag=f"k{c}", name=f"k{c}") for c in range(HC)]
    vt = [acts.tile([128, H], BF16, tag=f"v{j}", name=f"v{j}")
          for j in range(NT + 1)]

    # ---- projections ----
    for hc in range(HC):                      # qT [H, 2048]
        for t4 in range(TC // 512):
            ps = psP.tile([128, 512], F32, tag="ps", name="psq")
            for c in range(HC):
                nc.tensor.matmul(
                    ps[:], wqt[c][:, hc * 128:(hc + 1) * 128],
                    xt[c][:, HALO + t4 * 512: HALO + (t4 + 1) * 512],
                    start=(c == 0), stop=(c == HC - 1))
            nc.scalar.activation(
                qt[hc][:, t4 * 512:(t4 + 1) * 512], ps[:],
                AF.Identity, bias=bq_t[:, hc:hc + 1], scale=1.0)

    kchunks = [(i * 512, 512) for i in range(TC // 512)] + [(TC, HALO)]
    for hc in range(HC):                      # kT [H, 2056]
        for (off, sz) in kchunks:
            ps = psP.tile([128, 512], F32, tag="ps", name="psk")
            for c in range(HC):
                nc.tensor.matmul(
                    ps[:, :sz], wkt[c][:, hc * 128:(hc + 1) * 128],
                    xt[c][:, off:off + sz],
                    start=(c == 0), stop=(c == HC - 1))
            nc.scalar.activation(
                kt[hc][:, off:off + sz], ps[:, :sz],
                AF.Identity, bias=bk_t[:, hc:hc + 1], scale=1.0)

    for j in range(NT + 1):                   # v [2056, H] row-major
        pr = 128 if j < NT else HALO
        for hh in range(2):
            ps = psP.tile([128, 512], F32, tag="ps", name="psv")
            for c in range(HC):
                nc.tensor.matmul(
                    ps[:pr, :], xt[c][:, j * 128: j * 128 + pr],
                    wvt[c][:, hh * 512:(hh + 1) * 512],
                    start=(c == 0), stop=(c == HC - 1))
            nc.vector.tensor_add(
                vt[j][:pr, hh * 512:(hh + 1) * 512], ps[:pr, :],
                bv_t[:pr, hh * 512:(hh + 1) * 512])

    # ---- windowed attention ----
    for j in range(NT):
        s_ps = psS.tile([128, SPAN], F32, tag="s", name="s_ps")
        for c in range(HC):
            nc.tensor.matmul(
                s_ps[:], qt[c][:, j * 128:(j + 1) * 128],
                kt[c][:, j * 128: j * 128 + SPAN],
                start=(c == 0), stop=(c == HC - 1))
        s_sb = attn.tile([128, SPAN], F32, tag="ssb", name="s_sb")
        nc.vector.tensor_add(s_sb[:], s_ps[:],
                             (m0_t if j == 0 else mr_t)[:])
        negmax = attn.tile([128, 1], F32, tag="nm", name="negmax")
        nc.vector.reduce_max(negmax[:], s_sb[:],
                             axis=mybir.AxisListType.X, negate=True)
        p_bf = attn.tile([128, SPAN], BF16, tag="p", name="p_bf")
        rowsum = attn.tile([128, 1], F32, tag="rs", name="rowsum")
        nc.scalar.activation(p_bf[:], s_sb[:], AF.Exp,
                             bias=negmax[:], scale=1.0,
                             accum_out=rowsum[:])
        rinv = attn.tile([128, 1], F32, tag="ri", name="rinv")
        nc.vector.reciprocal(rinv[:], rowsum[:])

        pta_ps = psT.tile([128, 128], BF16, tag="pta", name="pta_ps")
        ptb_ps = psT.tile([HALO, 128], BF16, tag="ptb", name="ptb_ps")
        nc.tensor.transpose(pta_ps[:], p_bf[:, 0:128], id_t[:])
        nc.tensor.transpose(ptb_ps[:], p_bf[:, 128:SPAN], id_t[:])
        pta_sb = attn.tile([128, 128], BF16, tag="ptas", name="pta_sb")
        ptb_sb = attn.tile([HALO, 128], BF16, tag="ptbs", name="ptb_sb")
        nc.scalar.copy(pta_sb[:], pta_ps[:])
        nc.vector.tensor_copy(ptb_sb[:], ptb_ps[:])

        out_sb = attn.tile([128, H], F32, tag="osb", name="out_sb")
        for hh in range(2):
            o_ps = psO.tile([128, 512], F32, tag="o", name="o_ps")
            nc.tensor.matmul(o_ps[:], pta_sb[:],
                             vt[j][:, hh * 512:(hh + 1) * 512],
                             start=True, stop=False)
            nc.tensor.matmul(o_ps[:], ptb_sb[:],
                             vt[j + 1][:HALO, hh * 512:(hh + 1) * 512],
                             start=False, stop=True)
            nc.scalar.activation(
                out_sb[:, hh * 512:(hh + 1) * 512], o_ps[:],
                AF.Copy, bias=0.0, scale=rinv[:])
        nc.sync.dma_start(out_d[j * 128:(j + 1) * 128, :], out_sb[:])


def _build(loop_reps=None):
    key = ("nc", loop_reps)
    if key in _cache:
        return _cache[key]
    nc = bacc.Bacc("TRN2", target_bir_lowering=False, debug=False,
                   num_devices=NCORES)

    aps = (
        nc.dram_tensor("x", [H, TH], BF16, kind="ExternalInput").ap(),
        nc.dram_tensor("wq", [H, H], BF16, kind="ExternalInput").ap(),
        nc.dram_tensor("wk", [H, H], BF16, kind="ExternalInput").ap(),
        nc.dram_tensor("wv", [H, H], BF16, kind="ExternalInput").ap(),
        nc.dram_tensor("bq", [128, HC], F32, kind="ExternalInput").ap(),
        nc.dram_tensor("bk", [128, HC], F32, kind="ExternalInput").ap(),
        nc.dram_tensor("bv", [128, H], F32, kind="ExternalInput").ap(),
        nc.dram_tensor("m0", [128, SPAN], F32, kind="ExternalInput").ap(),
        nc.dram_tensor("mr", [128, SPAN], F32, kind="ExternalInput").ap(),
        nc.dram_tensor("ident", [128, 128], BF16, kind="ExternalInput").ap(),
        nc.dram_tensor("out", [TC, H], F32, kind="ExternalOutput").ap(),
    )

    with tile.TileContext(nc) as tc:
        with (
            tc.tile_pool(name="consts", bufs=1) as consts,
            tc.tile_pool(name="xw", bufs=1) as xw,
            tc.tile_pool(name="acts", bufs=1) as acts,
            tc.tile_pool(name="psP", bufs=2, space="PSUM") as psP,
            tc.tile_pool(name="psS", bufs=2, space="PSUM") as psS,
            tc.tile_pool(name="psT", bufs=1, space="PSUM") as psT,
            tc.tile_pool(name="psO", bufs=2, space="PSUM") as psO,
            tc.tile_pool(name="attn", bufs=3) as attn,
        ):
            pools = (consts, xw, acts, psP, psS, psT, psO, attn)
            if loop_reps:
                with tc.For_i(0, loop_reps, 1):
                    _emit(nc, tc, aps, pools)
            else:
                _emit(nc, tc, aps, pools)

    nc.compile()
    _cache[key] = nc
    return nc


def _host_inputs(states, Wq, bq, Wk, bk, Wv, bv):
    """Shared (per-run) host-side tensor prep."""
    scale = 1.0 / np.sqrt(H)
    wq_h = np.ascontiguousarray(np.asarray(Wq, np.float32).T * scale).astype(BF)
    wk_h = np.ascontiguousarray(np.asarray(Wk, np.float32).T).astype(BF)
    wv_h = np.ascontiguousarray(np.asarray(Wv, np.float32).T).astype(BF)
    bq_h = np.ascontiguousarray(
        (np.asarray(bq, np.float32) * scale).reshape(HC, 128).T)
    bk_h = np.ascontiguousarray(np.asarray(bk, np.float32).reshape(HC, 128).T)
    bv_h = np.ascontiguousarray(
        np.broadcast_to(np.asarray(bv, np.float32), (128, H)))
    m = np.arange(128)[:, None]
    n = np.arange(SPAN)[None, :]
    band = (n >= m) & (n <= m + HALO)
    mr_h = np.where(band, 0.0, -30000.0).astype(np.float32)
    m0_h = np.where(band & (n >= HALO), 0.0, -30000.0).astype(np.float32)
    id_h = np.eye(128).astype(BF)
    return wq_h, wk_h, wv_h, bq_h, bk_h, bv_h, m0_h, mr_h, id_h


def _shard_maps(states, hosts):
    wq_h, wk_h, wv_h, bq_h, bk_h, bv_h, m0_h, mr_h, id_h = hosts
    in_maps = []
    for i in range(NCORES):
        b, hf = i // 2, i % 2
        xs = np.zeros((TH, H), np.float32)
        if hf == 0:
            xs[HALO:] = states[b, 0:TC]
        else:
            xs[:] = states[b, TC - HALO: 2 * TC]
        in_maps.append({
            "x": np.ascontiguousarray(xs.T).astype(BF),
            "wq": wq_h, "wk": wk_h, "wv": wv_h,
            "bq": bq_h, "bk": bk_h, "bv": bv_h,
            "m0": (m0_h if hf == 0 else mr_h), "mr": mr_h, "ident": id_h,
        })
    return in_maps


def kernel(states, Wq, bq, Wk, bk, Wv, bv, window):
    assert int(window) == HALO
    states = np.asarray(states, np.float32)
    nc = _build()
    hosts = _host_inputs(states, Wq, bq, Wk, bk, Wv, bv)
    in_maps = _shard_maps(states, hosts)
    res = run_bass_kernel_spmd(nc, in_maps, list(range(NCORES)))
    out = np.empty((B, T, H), np.float32)
    for i in range(NCORES):
        b, hf = i // 2, i % 2
        out[b, hf * TC:(hf + 1) * TC] = res.results[i]["out"]
    return out
